# revision 1
# baseline (speedup 1.0000x reference)
"""Bass/Trainium2 kernel for nn_DeformMaxPool2d.

Reference op: x [16,64,256,256] f32, gather_idx [128,128,4] int64 (an exact
permutation of 0..65535 pixel indices). out[b,c,i,j] = max_k x_flat[b,c,idx[i,j,k]].

Strategy (8 NeuronCores, data-parallel over the 1024 (b,c) pairs):
  - Each core owns 128 (b,c) pairs. Host transposes its slice to pixel-major
    xt [65536 pixels, 128 pairs] so one pixel's 128 pair-values are a
    contiguous 512B row — the DMA-gather sweet spot.
  - Device: dma_gather (SWDGE Ant ucode) pulls rows xt[idx,:] into SBUF,
    DVE max-reduces the 4 sources per output position, result rows are
    DMA'd back to HBM. Host re-permutes rows and reassembles the output.

dma_gather indices are int16, so rows are addressed within two 32768-row
halves of xt. Each output's 4 sources split between halves; outputs are
grouped host-side into classes by low-half source count l (0..4) so every
gather list is fully valid (no -1 padding mid-list, which the ucode does
not support). Per chunk of 1024 same-class outputs:
  gather A: the l low-half sources/output   -> gA [128, 8*l, 128]
  gather B: the 4-l high-half sources/output-> gB [128, 8*(4-l), 128]
  reduce A, reduce B (DVE max over sources), tensor_max(A,B) -> o
  DMA o to out rows; host inverts the class-sort at the end.
Gather list order j = (i*K' + k)*128 + p lands row j at partition j%128,
slot j//128 (snake), so output (p,i)'s sources sit at slots i*K'+k — a
fixed-stride view the DVE reduce consumes directly.
"""
import sys
sys.path.insert(0, '/opt/trn_rl_repo')

import numpy as np

B, C, D = 16, 64, 256
HO = 128
K = 4
P = 128
NCORES = 8
NPIX = D * D            # 65536
NOUT = HO * HO          # 16384
PAIRS = B * C           # 1024
PAIRS_PER_CORE = PAIRS // NCORES  # 128
HALF = NPIX // 2        # 32768: int16 index range for dma_gather

N_PER_CHUNK = 2         # outputs per partition per chunk (512-idx gathers: conservative vs ucode size limits)
CH = P * N_PER_CHUNK    # 1024 outputs per chunk
BUFS = 3


def _wrap16(lst):
    """dma_gather index layout: element j -> partition j%16, col j//16,
    replicated across the 8 gpsimd groups -> [128, len/16] int16."""
    a = np.asarray(lst, np.int16).reshape(-1, 16).T       # [16, len/16]
    return np.tile(a, (8, 1))


def make_plan(gather_idx):
    """Host planning: class-sort outputs, build gather lists + chunk table.

    Returns (idx_dev [128, totcols] int16, plan, out_perm, tot_rows):
      plan: per chunk (l, aoff, acols, boff, bcols, nA, nB)
      out_perm[t] = real output id for device out row t (or -1 for padding)
    """
    n = N_PER_CHUNK
    g4 = np.asarray(gather_idx).reshape(NOUT, K).astype(np.int64)
    lcnt = (g4 < HALF).sum(axis=1)                        # [NOUT] 0..4
    cols_blocks = []
    plan = []
    out_perm = []
    col = 0
    for l in range(K + 1):
        ids = np.nonzero(lcnt == l)[0]
        if len(ids) == 0:
            continue
        npad = (-len(ids)) % CH
        # dummy outputs: sources all 0 (low) / HALF (high) to stay in-range,
        # matching class l's split; their results are dropped host-side.
        ids_p = np.concatenate([ids, np.full(npad, -1, np.int64)])
        dummy = np.array([0] * l + [HALF] * (K - l), np.int64)
        for c0 in range(0, len(ids_p), CH):
            blk = ids_p[c0:c0 + CH]                       # [CH] output ids
            rows = np.empty((CH, K), np.int64)
            real = blk >= 0
            rows[real] = g4[blk[real]]
            rows[~real] = dummy
            # per output, its sources sorted so the l low ones come first
            order = np.argsort(rows >= HALF, axis=1, kind="stable")
            rows = np.take_along_axis(rows, order, axis=1)
            low = rows[:, :l]                             # [CH, l]
            high = rows[:, l:] - HALF                     # [CH, 4-l]
            # list order j = (i*K' + k)*128 + p ; output (p,i) = blk[p*n+i]
            # blk index q = p*n+i -> i = q%n, p = q//n
            def mklist(src):                              # src [CH, K']
                kk = src.shape[1]
                s = src.reshape(P, n, kk)                 # [p, i, k]
                return s.transpose(1, 2, 0).reshape(-1)   # j = (i*kk+k)*128+p
            entry = [l, 0, 0, 0, 0, n * l * P, n * (K - l) * P]
            if l > 0:
                la = mklist(low)
                wa = _wrap16(la)
                entry[1], entry[2] = col, wa.shape[1]
                cols_blocks.append(wa)
                col += wa.shape[1]
            if l < K:
                lb = mklist(high)
                wb = _wrap16(lb)
                entry[3], entry[4] = col, wb.shape[1]
                cols_blocks.append(wb)
                col += wb.shape[1]
            plan.append(tuple(entry))
            out_perm.append(blk)
    idx_dev = np.ascontiguousarray(np.concatenate(cols_blocks, axis=1))
    out_perm = np.concatenate(out_perm)                   # [tot_rows]
    return idx_dev, plan, out_perm, len(out_perm)


def build_program(plan, totcols, tot_rows, repeats=1, bufs=BUFS):
    import concourse.bass as bass
    import concourse.bacc as bacc
    import concourse.tile as tile
    from concourse import mybir

    n = N_PER_CHUNK
    nc = bacc.Bacc("TRN2")
    xlo_d = nc.dram_tensor("xlo", [HALF, P], mybir.dt.float32, kind="ExternalInput")
    xhi_d = nc.dram_tensor("xhi", [NPIX - HALF, P], mybir.dt.float32,
                           kind="ExternalInput")
    idx_d = nc.dram_tensor("idx", [P, totcols], mybir.dt.int16, kind="ExternalInput")
    out_d = nc.dram_tensor("out", [tot_rows, P], mybir.dt.float32,
                           kind="ExternalOutput")

    with tile.TileContext(nc) as tc:
        with tc.tile_pool(name="sbuf", bufs=1) as ipool, \
             tc.tile_pool(name="g", bufs=bufs) as gpool, \
             tc.tile_pool(name="r", bufs=bufs) as rpool, \
             tc.tile_pool(name="o", bufs=bufs) as opool:
            idx_t = ipool.tile([P, totcols], mybir.dt.int16)
            nc.sync.dma_start(out=idx_t[:], in_=idx_d[:])
            out_view = out_d[:].rearrange("(c p n) d -> c p n d", p=P, n=n)
            for _ in range(repeats):
                for ci, (l, aoff, acols, boff, bcols, nA, nB) in enumerate(plan):
                    o = opool.tile([P, n, P], mybir.dt.float32, tag="o")
                    rA = rB = None
                    if l > 0:
                        gA = gpool.tile([P, n * l, P], mybir.dt.float32, tag="gA")
                        nc.gpsimd.dma_gather(
                            gA[:], xlo_d[:], idx_t[:, aoff:aoff + acols],
                            nA, nA, P,
                        )
                        dstA = o if l == K else rpool.tile(
                            [P, n, P], mybir.dt.float32, tag="rA")
                        nc.vector.tensor_reduce(
                            out=dstA[:],
                            in_=gA[:].rearrange("p (n k) d -> p n d k", k=l),
                            axis=mybir.AxisListType.X,
                            op=mybir.AluOpType.max,
                        )
                        rA = dstA
                    if l < K:
                        gB = gpool.tile([P, n * (K - l), P], mybir.dt.float32,
                                        tag="gB")
                        nc.gpsimd.dma_gather(
                            gB[:], xhi_d[:], idx_t[:, boff:boff + bcols],
                            nB, nB, P,
                        )
                        dstB = o if l == 0 else rpool.tile(
                            [P, n, P], mybir.dt.float32, tag="rB")
                        nc.vector.tensor_reduce(
                            out=dstB[:],
                            in_=gB[:].rearrange("p (n k) d -> p n d k", k=K - l),
                            axis=mybir.AxisListType.X,
                            op=mybir.AluOpType.max,
                        )
                        rB = dstB
                    if 0 < l < K:
                        nc.vector.tensor_max(o[:], rA[:], rB[:])
                    nc.sync.dma_start(out=out_view[ci], in_=o[:])
    nc.compile()
    return nc


def shard_inputs(x):
    xf = np.asarray(x).reshape(PAIRS, NPIX)
    shards = []
    for j in range(NCORES):
        sl = xf[j * PAIRS_PER_CORE:(j + 1) * PAIRS_PER_CORE]
        xt = np.ascontiguousarray(sl.T)                    # [NPIX, 128]
        shards.append((xt[:HALF], np.ascontiguousarray(xt[HALF:])))
    return shards


def assemble_output(results, out_perm):
    full = np.empty((PAIRS, NOUT), np.float32)
    valid = out_perm >= 0
    perm = out_perm[valid]
    for j, r in enumerate(results):
        dev = np.asarray(r["out"])                         # [tot_rows, 128]
        full[j * PAIRS_PER_CORE:(j + 1) * PAIRS_PER_CORE, perm] = dev[valid].T
    return np.ascontiguousarray(full.reshape(B, C, HO, HO))


_cache = {}


def prepare(gather_idx, repeats=1):
    key = ("plan", gather_idx.shape, int(np.asarray(gather_idx)[0, 0, 0]),
           repeats)
    if key not in _cache:
        idx_dev, plan, out_perm, tot_rows = make_plan(gather_idx)
        nc = build_program(plan, idx_dev.shape[1], tot_rows, repeats=repeats)
        _cache[key] = (idx_dev, nc, out_perm)
    return _cache[key]


def kernel(x, gather_idx):
    from concourse.bass_utils import run_bass_kernel_spmd
    idx_dev, nc, out_perm = prepare(gather_idx)
    in_maps = [{"xlo": lo, "xhi": hi, "idx": idx_dev}
               for lo, hi in shard_inputs(x)]
    res = run_bass_kernel_spmd(nc, in_maps, list(range(NCORES)))
    return assemble_output(res.results, out_perm)

